# revision 1
# baseline (speedup 1.0000x reference)
"""GGNN layer (gated graph NN message passing) on Trainium2 via Bass/Tile.

Data-parallel over the batch dim: 64 graphs -> 8 NeuronCores x 8 graphs.
Each core runs an identical NEFF on its batch shard; weights are replicated.

Math per core, per graph b (N=512 nodes, D=512 features, steps=2):
    h = relu(x @ W_enc) * mask
    repeat 2x:
        a  = adj @ h
        z  = relu(a @ Wz + h @ Uz)
        r  = relu(a @ Wr + h @ Ur)
        hc = tanh(a @ Wh + (r*h) @ Uh) * mask
        h  = (1-z)*h + z*hc

All seven 512^3 matmuls per step run on the PE in fp8 (e4m3) DoubleRow mode
(2 contraction rows per PE cell), with per-tensor power-of-2 scales chosen
for the spec'd input distribution. Accuracy-critical matmuls use a 3-term
hi/lo decomposition: for operands A ~ Ahi+Alo, B ~ Bhi+Blo (each fp8 with a
shared scale), A@B ~ Ahi@Bhi + Ahi@Blo + Alo@Bhi accumulated in fp32 PSUM
(residual ~2^-8 relative, ~1e-3 end to end). Error-tolerant matmuls (the
U-side gate products and most of step 2, where tanh saturation and the
z-dominated combine squash quantization noise) use the single hi@hi term.
x and adj are scaled/split/transposed on the host; activations are
quantized on chip (ACT produces the scaled value, Pool rounds hi, DVE
computes the lo residual). Aggregated messages `a` stay fp32 before
quantization; gates/state use bf16 for 2x DVE throughput. Biases are zero
and mask is all-ones per the problem spec (host fallback handles anything
else bit-exactly via numpy).

Layout: activations feature-major [d_part, 4 k-tiles x 512] like the
matmul contraction wants; node-major copies for the adjacency matmul are
made with fp8 PE transposes (stride-2 PSUM writes). The stride-2 layout is
kept all the way into SBUF — the PSUM->SBUF move is a bitcast uint32 copy
(4x fewer elements) and the a-matmuls read their stationary operand with
inner stride 2. The three phases per graph (encode / step 1 / step 2) are
software-pipelined across graphs: slot t interleaves step2(t-2),
step1(t-1), encode(t) so PE, ACT, DVE and Pool all stay busy.
"""

import numpy as np

B, NN, DD = 64, 512, 512
P = 128
KT = DD // P
NCORES = 8
B_PC = B // NCORES

_BUILT = {}
LAST_RESULTS = None

# ---- scales (powers of two, tuned for the spec'd input distribution with
# ~2x headroom under the e4m3 max of 240). The h0 state is stored scaled by
# S_H0 (its fp8 lo residual needs the scaled tensor); h1 is stored unscaled
# and S_H1 is applied only when quantizing its fp8 hi copy. ----
S_X = 16.0
S_ADJ = 64.0
S_H0 = 16.0
S_H1 = 2.0 ** -5
S_A1 = 2.0 ** -1
S_A2 = 2.0 ** -12
S_RH1 = 2.0 ** -5
S_RH2 = 2.0 ** -19
S_WENC = 512.0
S_WZ = 512.0
S_WR = 512.0
S_WH1 = 64.0
S_WH2 = 8.0
# U-side scales are pinned by the shared-PSUM constraint S_a*S_w == S_h*S_u
S_UZ1 = S_A1 * S_WZ / S_H0      # 16
S_UZ2 = S_A2 * S_WZ / S_H1      # 4
S_UR1 = S_A1 * S_WR / S_H0      # 16
S_UR2 = S_A2 * S_WR / S_H1      # 4
S_UH1 = S_A1 * S_WH1 / S_RH1    # 1024
S_UH2 = S_A2 * S_WH2 / S_RH2    # 1024


def _build():
    from contextlib import ExitStack
    import concourse.bacc as bacc
    import concourse.tile as tile
    import concourse.mybir as mybir

    FP = mybir.dt.float32
    BF = mybir.dt.bfloat16
    F8 = mybir.dt.float8e4
    ACT = mybir.ActivationFunctionType
    DR = mybir.MatmulPerfMode.DoubleRow

    nc = bacc.Bacc("TRN2", target_bir_lowering=False, debug=False,
                   num_devices=NCORES)

    TDD = KT * DD
    xhi_d = nc.dram_tensor("xhi", [B_PC, P, TDD], F8, kind="ExternalInput").ap()
    xlo_d = nc.dram_tensor("xlo", [B_PC, P, TDD], F8, kind="ExternalInput").ap()
    ahi_d = nc.dram_tensor("adjhi", [B_PC, P, TDD], F8, kind="ExternalInput").ap()
    alo_d = nc.dram_tensor("adjlo", [B_PC, P, TDD], F8, kind="ExternalInput").ap()
    WNAMES = ["wenchi", "wenclo", "wzhi", "wzlo", "wrhi", "wrlo",
              "wh1hi", "wh1lo", "uh1hi", "uh1lo",
              "uz1hi", "ur1hi", "uz2hi", "ur2hi", "wh2hi", "uh2hi"]
    w_d = {n: nc.dram_tensor(n, [P, TDD], F8, kind="ExternalInput").ap()
           for n in WNAMES}
    out_d = nc.dram_tensor("out", [B_PC, DD, NN], FP, kind="ExternalOutput").ap()

    with tile.TileContext(nc) as tc:
        with ExitStack() as ctx:
            consts = ctx.enter_context(tc.tile_pool(name="consts", bufs=1))
            xpool = ctx.enter_context(tc.tile_pool(name="x", bufs=3))
            adjpool = ctx.enter_context(tc.tile_pool(name="adj", bufs=4))
            hpool = ctx.enter_context(tc.tile_pool(name="h", bufs=4))
            hhipool = ctx.enter_context(tc.tile_pool(name="hhi", bufs=4))
            hlopool = ctx.enter_context(tc.tile_pool(name="hlo", bufs=2))
            nmhipool = ctx.enter_context(tc.tile_pool(name="nmhi", bufs=4))
            nmlopool = ctx.enter_context(tc.tile_pool(name="nmlo", bufs=2))
            atpool = ctx.enter_context(tc.tile_pool(name="at", bufs=2))
            a8pool = ctx.enter_context(tc.tile_pool(name="a8", bufs=4))
            zpool = ctx.enter_context(tc.tile_pool(name="z", bufs=2))
            rpool = ctx.enter_context(tc.tile_pool(name="r", bufs=2))
            rhpool = ctx.enter_context(tc.tile_pool(name="rh", bufs=2))
            rh8pool = ctx.enter_context(tc.tile_pool(name="rh8", bufs=3))
            hcpool = ctx.enter_context(tc.tile_pool(name="hc", bufs=2))
            upool = ctx.enter_context(tc.tile_pool(name="u", bufs=2))
            scpool = ctx.enter_context(tc.tile_pool(name="sc", bufs=6))
            outpool = ctx.enter_context(tc.tile_pool(name="outp", bufs=2))
            mmps = ctx.enter_context(tc.tile_pool(name="mmps", bufs=4, space="PSUM"))
            tps = ctx.enter_context(tc.tile_pool(name="tps", bufs=4, space="PSUM"))

            # fp8 identity for PE transposes, built on chip
            idf = consts.tile([P, P], FP, tag="idf")
            nc.gpsimd.memset(idf[:], 1.0)
            nc.gpsimd.affine_select(idf[:], idf[:], pattern=[[-1, P]],
                                    compare_op=mybir.AluOpType.is_equal,
                                    fill=0.0, channel_multiplier=1)
            id8 = consts.tile([P, P], F8, tag="id8")
            nc.vector.tensor_copy(id8[:], idf[:])

            # PE warmup during the first DMAs so real work starts ramped
            warm = tps.tile([P, 2 * P], F8, tag="tps")
            warm_v = warm[:].rearrange("p (d two) -> p d two", two=2)[:, :, 0:1] \
                .rearrange("p d one -> p (d one)")
            for _ in range(24):
                nc.tensor.transpose(warm_v, id8[:], id8[:])

            # ---- weights: batch-0 x/adj first, then by first use ----
            w_sb = {}

            def loadw(n):
                t = consts.tile([P, TDD], F8, tag=f"w_{n}")
                nc.sync.dma_start(t[:], w_d[n])
                w_sb[n] = t

            def pairs(t):
                return t[:].rearrange("p (k d) -> p k d", k=KT)

            def mm(ps_ap, wt, act, pp, first, last):
                nc.tensor.matmul(
                    ps_ap,
                    wt, act[:, 2 * pp:2 * pp + 2, :],
                    start=first, stop=last, perf_mode=DR,
                )

            def gate_group(ps, ej, terms):
                """terms: list of (w_tile, act_pairs_ap). 2 pair-instrs each."""
                n = len(terms) * 2
                i = 0
                for wt, act in terms:
                    wp = pairs(wt)
                    for pp in range(2):
                        mm(ps[:], wp[:, 2 * pp:2 * pp + 2, ej * P:(ej + 1) * P],
                           act, pp, i == 0, i == n - 1)
                        i += 1

            U32 = mybir.dt.uint32

            def transpose_g(dst_sb, src_sb, nj, copy_eng):
                """Transpose column-block nj of fp8 fm tile src into nm dst.

                The fp8 transpose writes PSUM at element step 2; the stride-2
                layout is moved to SBUF verbatim as a bitcast uint32 copy (4x
                fewer elements than an fp8 compaction) and the a-matmuls read
                the nm operand with inner stride 2."""
                pt_t = tps.tile([P, 2 * DD], F8, tag="tps")
                pt = pt_t[:]
                ptv = pt.rearrange("p (d two) -> p d two", two=2)[:, :, 0:1] \
                    .rearrange("p d one -> p (d one)")
                for ib in range(KT):
                    nc.tensor.transpose(
                        ptv[:, ib * P:(ib + 1) * P],
                        src_sb[:, ib * DD + nj * P: ib * DD + (nj + 1) * P],
                        id8[:],
                    )
                dst = dst_sb[:, nj * 2 * DD:(nj + 1) * 2 * DD]
                if copy_eng == "act":
                    nc.scalar.copy(dst.bitcast(U32), pt.bitcast(U32))
                else:
                    nc.vector.tensor_copy(dst.bitcast(U32), pt.bitcast(U32))

            # ---------------- phases ----------------
            def dma_in(b, st):
                """Input DMAs for batch b (emitted one slot ahead)."""
                def f():
                    xhi = xpool.tile([P, TDD], F8, tag="xhi")
                    xlo = xpool.tile([P, TDD], F8, tag="xlo")
                    adjhi = adjpool.tile([P, TDD], F8, tag="adjhi")
                    adjlo = adjpool.tile([P, TDD], F8, tag="adjlo")
                    nc.sync.dma_start(xhi[:], xhi_d[b])
                    nc.sync.dma_start(xlo[:], xlo_d[b])
                    nc.sync.dma_start(adjhi[:], ahi_d[b])
                    nc.sync.dma_start(adjlo[:], alo_d[b])
                    st.update(xhi=xhi, xlo=xlo, adjhi=adjhi, adjlo=adjlo)
                return f

            def p0_chunks(b, st):
                """Encode batch b: enc matmul, H0 + hi/lo + nm transposes."""
                ch = []

                H0 = hpool.tile([P, TDD], BF, tag="h")
                H0hi = hhipool.tile([P, TDD], F8, tag="hhi")
                H0lo = hlopool.tile([P, TDD], F8, tag="hlo")
                st.update(H=H0, Hhi=H0hi, Hlo=H0lo)

                def enc_ej(ej):
                    def f():
                        ps = mmps.tile([P, DD], FP, tag="mmps")
                        xh, xl = pairs(st["xhi"]), pairs(st["xlo"])
                        gate_group(ps, ej, [(w_sb["wenchi"], xh),
                                            (w_sb["wenclo"], xh),
                                            (w_sb["wenchi"], xl)])
                        nc.scalar.activation(H0[:, ej * DD:(ej + 1) * DD], ps[:],
                                             ACT.Relu, scale=S_H0 / (S_X * S_WENC))
                    return f
                for ej in range(KT):
                    ch.append(enc_ej(ej))

                def hi_half(h):
                    def f():
                        s = slice(h * 2 * DD, (h + 1) * 2 * DD)
                        nc.gpsimd.tensor_copy(H0hi[:, s], H0[:, s])
                    return f
                ch.append(hi_half(0))
                ch.append(hi_half(1))

                def lo_ej(ej):
                    def f():
                        s = slice(ej * DD, (ej + 1) * DD)
                        nc.vector.tensor_sub(H0lo[:, s], H0[:, s], H0hi[:, s])
                    return f
                for ej in range(KT):
                    ch.append(lo_ej(ej))

                nmhi = nmhipool.tile([P, 2 * TDD], F8, tag="nmhi")
                nmlo = nmlopool.tile([P, 2 * TDD], F8, tag="nmlo")
                st.update(nmhi=nmhi, nmlo=nmlo)
                for nj in range(KT):
                    ch.append(lambda nj=nj: transpose_g(
                        nmhi, H0hi, nj, "act" if nj % 2 == 0 else "dve"))
                for nj in range(KT):
                    ch.append(lambda nj=nj: transpose_g(
                        nmlo, H0lo, nj, "act" if nj % 2 == 1 else "dve"))
                return ch

            def nm_pairs(t):
                """Pairs view of a stride-2 nm tile: [p, k, d] with d-stride 2."""
                return t[:].rearrange("p (k d two) -> p k d two", k=KT, two=2) \
                    [:, :, :, 0:1]

            def amm_terms(st, single):
                """DoubleRow terms for a = adj @ h (contraction over nodes)."""
                adjh, adjl = pairs(st["adjhi"]), pairs(st["adjlo"])
                nmh, nml = st["nmhi"], st["nmlo"]
                if single:
                    return [(nmh, adjh)]
                return [(nmh, adjh), (nmh, adjl), (nml, adjh)]

            def a_group(ps, di, st, single):
                terms = amm_terms(st, single)
                n = len(terms) * 2
                i = 0
                for nmt, act in terms:
                    nmp = nm_pairs(nmt)
                    for pp in range(2):
                        nc.tensor.matmul(
                            ps[:],
                            nmp[:, 2 * pp:2 * pp + 2, di * P:(di + 1) * P, :]
                            .rearrange("p k d one -> p k (d one)"),
                            act[:, 2 * pp:2 * pp + 2, :],
                            start=(i == 0), stop=(i == n - 1), perf_mode=DR,
                        )
                        i += 1

            def p1_chunks(b, st):
                """Step 1 on batch b (state S_H0*h0 in -> unscaled h1 out)."""
                ch = []
                at = atpool.tile([P, TDD], FP, tag="at")
                ahi = a8pool.tile([P, TDD], F8, tag="ahi")
                alo = a8pool.tile([P, TDD], F8, tag="alo")
                H0 = st["H"]

                def a_di(di):
                    def f():
                        ps = mmps.tile([P, DD], FP, tag="mmps")
                        a_group(ps, di, st, single=False)
                        s = slice(di * DD, (di + 1) * DD)
                        nc.scalar.activation(at[:, s], ps[:], ACT.Copy,
                                             scale=S_A1 / (S_H0 * S_ADJ))
                    return f

                def aq_di(di):
                    def f():
                        s = slice(di * DD, (di + 1) * DD)
                        nc.gpsimd.tensor_copy(ahi[:, s], at[:, s])
                        nc.vector.tensor_sub(alo[:, s], at[:, s], ahi[:, s])
                    return f
                for di in range(KT):
                    ch.append(a_di(di))
                    ch.append(aq_di(di))

                # u = h0 (unscaled) — exact power-of-2 rescale
                u = upool.tile([P, TDD], BF, tag="u")
                ch.append(lambda: nc.vector.tensor_scalar_mul(u[:], H0[:], 1.0 / S_H0))

                zs = zpool.tile([P, TDD], BF, tag="z")
                rs = rpool.tile([P, TDD], BF, tag="r")
                ap_, al_ = pairs(ahi), pairs(alo)
                hp_ = pairs(st["Hhi"])

                def z_ej(ej):
                    def f():
                        ps = mmps.tile([P, DD], FP, tag="mmps")
                        gate_group(ps, ej, [(w_sb["wzhi"], ap_), (w_sb["wzlo"], ap_),
                                            (w_sb["wzhi"], al_), (w_sb["uz1hi"], hp_)])
                        nc.scalar.activation(zs[:, ej * DD:(ej + 1) * DD], ps[:],
                                             ACT.Relu, scale=1.0 / (S_A1 * S_WZ))
                    return f

                rh = rhpool.tile([P, TDD], BF, tag="rh")
                rhhi = rh8pool.tile([P, TDD], F8, tag="rhhi")
                rhlo = rh8pool.tile([P, TDD], F8, tag="rhlo")

                def r_ej(ej):
                    def f():
                        ps = mmps.tile([P, DD], FP, tag="mmps")
                        gate_group(ps, ej, [(w_sb["wrhi"], ap_), (w_sb["wrlo"], ap_),
                                            (w_sb["wrhi"], al_), (w_sb["ur1hi"], hp_)])
                        s = slice(ej * DD, (ej + 1) * DD)
                        nc.scalar.activation(rs[:, s], ps[:], ACT.Relu,
                                             scale=S_RH1 / (S_A1 * S_WR * S_H0))
                        nc.vector.tensor_mul(rh[:, s], rs[:, s], H0[:, s])
                    return f

                def rhq_ej(ej):
                    def f():
                        s = slice(ej * DD, (ej + 1) * DD)
                        nc.gpsimd.tensor_copy(rhhi[:, s], rh[:, s])
                        nc.vector.tensor_sub(rhlo[:, s], rh[:, s], rhhi[:, s])
                    return f
                for ej in range(KT):
                    ch.append(z_ej(ej))
                    ch.append(r_ej(ej))
                for ej in range(KT):
                    ch.append(rhq_ej(ej))

                hc = hcpool.tile([P, TDD], BF, tag="hc")
                rhp_, rlp_ = pairs(rhhi), pairs(rhlo)

                def hc_ej(ej):
                    def f():
                        ps = mmps.tile([P, DD], FP, tag="mmps")
                        gate_group(ps, ej, [(w_sb["wh1hi"], ap_), (w_sb["wh1lo"], ap_),
                                            (w_sb["wh1hi"], al_), (w_sb["uh1hi"], rhp_),
                                            (w_sb["uh1lo"], rhp_), (w_sb["uh1hi"], rlp_)])
                        nc.scalar.activation(hc[:, ej * DD:(ej + 1) * DD], ps[:],
                                             ACT.Tanh, scale=1.0 / (S_A1 * S_WH1))
                    return f
                for ej in range(KT):
                    ch.append(hc_ej(ej))

                H1 = hpool.tile([P, TDD], BF, tag="h")
                H1hi = hhipool.tile([P, TDD], F8, tag="hhi")

                def comb_ej(ej):
                    def f():
                        s = slice(ej * DD, (ej + 1) * DD)
                        t1 = scpool.tile([P, DD], BF, tag="sc")
                        w_ = scpool.tile([P, DD], BF, tag="sc")
                        t3 = scpool.tile([P, DD], BF, tag="sc")
                        nc.vector.tensor_mul(t1[:], zs[:, s], u[:, s])
                        nc.vector.tensor_sub(w_[:], u[:, s], t1[:])
                        nc.vector.tensor_mul(t3[:], zs[:, s], hc[:, s])
                        nc.vector.tensor_add(H1[:, s], w_[:], t3[:])
                    return f
                for ej in range(KT):
                    ch.append(comb_ej(ej))

                def h1hi_half(h):
                    def f():
                        s = slice(h * 2 * DD, (h + 1) * 2 * DD)
                        nc.gpsimd.tensor_scalar_mul(H1hi[:, s], H1[:, s], S_H1)
                    return f
                ch.append(h1hi_half(0))
                ch.append(h1hi_half(1))

                nmhi = nmhipool.tile([P, 2 * TDD], F8, tag="nmhi")
                for nj in range(KT):
                    ch.append(lambda nj=nj: transpose_g(
                        nmhi, H1hi, nj, "act" if nj % 2 == 0 else "dve"))

                def fin():
                    st.update(H=H1, Hhi=H1hi, nmhi=nmhi)
                ch.append(fin)
                return ch

            def p2_chunks(b, st):
                """Step 2 on batch b + output stores (unscaled fp32 out)."""
                ch = []
                at = atpool.tile([P, TDD], FP, tag="at")
                ahi = a8pool.tile([P, TDD], F8, tag="ahi")
                alo = a8pool.tile([P, TDD], F8, tag="alo")

                def a_di(di):
                    def f():
                        ps = mmps.tile([P, DD], FP, tag="mmps")
                        a_group(ps, di, st, single=True)
                        s = slice(di * DD, (di + 1) * DD)
                        nc.scalar.activation(at[:, s], ps[:], ACT.Copy,
                                             scale=S_A2 / (S_H1 * S_ADJ))
                    return f

                def aq_di(di):
                    def f():
                        s = slice(di * DD, (di + 1) * DD)
                        nc.gpsimd.tensor_copy(ahi[:, s], at[:, s])
                        nc.vector.tensor_sub(alo[:, s], at[:, s], ahi[:, s])
                    return f
                for di in range(KT):
                    ch.append(a_di(di))
                    ch.append(aq_di(di))

                H1 = st["H"]
                z = zpool.tile([P, TDD], BF, tag="z")
                rs = rpool.tile([P, TDD], BF, tag="r")
                rhhi = rh8pool.tile([P, TDD], F8, tag="rhhi")
                hc = hcpool.tile([P, TDD], BF, tag="hc")
                ap_, al_ = pairs(ahi), pairs(alo)
                hp_ = pairs(st["Hhi"])

                def z_ej(ej):
                    def f():
                        ps = mmps.tile([P, DD], FP, tag="mmps")
                        gate_group(ps, ej, [(w_sb["wzhi"], ap_), (w_sb["wzlo"], ap_),
                                            (w_sb["wzhi"], al_), (w_sb["uz2hi"], hp_)])
                        nc.scalar.activation(z[:, ej * DD:(ej + 1) * DD], ps[:],
                                             ACT.Relu, scale=1.0 / (S_A2 * S_WZ))
                    return f

                def r_ej(ej):
                    def f():
                        ps = mmps.tile([P, DD], FP, tag="mmps")
                        gate_group(ps, ej, [(w_sb["wrhi"], ap_), (w_sb["ur2hi"], hp_)])
                        s = slice(ej * DD, (ej + 1) * DD)
                        nc.scalar.activation(rs[:, s], ps[:], ACT.Relu,
                                             scale=S_RH2 / (S_A2 * S_WR))
                        nc.vector.tensor_mul(rhhi[:, s], rs[:, s], H1[:, s])
                    return f
                for ej in range(KT):
                    ch.append(z_ej(ej))
                    ch.append(r_ej(ej))

                rhp_ = pairs(rhhi)

                def hc_ej(ej):
                    def f():
                        ps = mmps.tile([P, DD], FP, tag="mmps")
                        gate_group(ps, ej, [(w_sb["wh2hi"], ap_), (w_sb["uh2hi"], rhp_)])
                        nc.scalar.activation(hc[:, ej * DD:(ej + 1) * DD], ps[:],
                                             ACT.Tanh, scale=1.0 / (S_A2 * S_WH2))
                    return f
                for ej in range(KT):
                    ch.append(hc_ej(ej))

                def comb_ej(ej):
                    def f():
                        s = slice(ej * DD, (ej + 1) * DD)
                        ot = outpool.tile([P, DD], FP, tag="outp")
                        t1 = scpool.tile([P, DD], BF, tag="sc")
                        w_ = scpool.tile([P, DD], BF, tag="sc")
                        t3 = scpool.tile([P, DD], BF, tag="sc")
                        nc.gpsimd.tensor_mul(t1[:], z[:, s], H1[:, s])
                        nc.vector.tensor_sub(w_[:], H1[:, s], t1[:])
                        nc.vector.tensor_mul(t3[:], z[:, s], hc[:, s])
                        nc.vector.tensor_add(ot[:], w_[:], t3[:])
                        nc.sync.dma_start(out_d[b, ej * P:(ej + 1) * P, :], ot[:])
                    return f
                for ej in range(KT):
                    ch.append(comb_ej(ej))
                return ch

            # ---- weight DMAs in first-use order ----
            for n in ["wenchi", "wenclo"]:
                loadw(n)

            def late_weights():
                for n in ["wzhi", "wzlo", "uz1hi", "wrhi", "wrlo", "ur1hi",
                          "wh1hi", "wh1lo", "uh1hi", "uh1lo",
                          "uz2hi", "ur2hi", "wh2hi", "uh2hi"]:
                    loadw(n)

            # ---- 3-phase pipeline: slot t = [P2(t-2), P1(t-1), P0(t)] ----
            def emit_slot(lists):
                # proportional round-robin merge, preserving per-list order
                tagged = []
                for li, lst in enumerate(lists):
                    n = len(lst)
                    for i, f in enumerate(lst):
                        tagged.append(((i + 0.5) / n, li, f))
                tagged.sort(key=lambda t: (t[0], t[1]))
                for _, _, f in tagged:
                    f()

            sts = [dict() for _ in range(B_PC)]
            dma_in(0, sts[0])()
            first = p0_chunks(0, sts[0])
            for f in first:
                f()
            dma_in(1, sts[1])()
            late_weights()
            for t in range(1, B_PC + 2):
                lists = []
                if 0 <= t - 2 < B_PC:
                    lists.append(p2_chunks(t - 2, sts[t - 2]))
                if 0 <= t - 1 < B_PC:
                    lists.append(p1_chunks(t - 1, sts[t - 1]))
                if t < B_PC:
                    lists.append(p0_chunks(t, sts[t]))
                emit_slot(lists)
                if t + 1 < B_PC:
                    dma_in(t + 1, sts[t + 1])()

    nc.compile()
    return nc


def _get():
    if "nc" not in _BUILT:
        _BUILT["nc"] = _build()
    return _BUILT["nc"]


def _lay(M):
    """[512, 512] (contraction-major) -> [128, 2048] SBUF tile layout."""
    return np.ascontiguousarray(
        M.reshape(KT, P, DD).transpose(1, 0, 2).reshape(P, KT * DD))


def _split8(M, scale):
    import ml_dtypes
    E4 = ml_dtypes.float8_e4m3
    s = (M * scale).astype(np.float32)
    hi = s.astype(E4)
    lo = (s - hi.astype(np.float32)).astype(E4)
    return hi, lo


def _lay_batch(A):
    """[B_PC, 512, 512] fp8, transpose each graph then tile layout."""
    t = A.transpose(0, 2, 1)
    return np.ascontiguousarray(
        t.reshape(B_PC, KT, P, DD).transpose(0, 2, 1, 3).reshape(B_PC, P, KT * DD))


def _fallback(x, adj, mask, W_enc, b_enc, Wz, Uz, bz, Wr, Ur, br, Wh, Uh, bh,
              ba, steps):
    h = mask * np.maximum(x @ W_enc + b_enc, 0.0)
    for _ in range(steps):
        a = np.einsum("bnm,bmd->bnd", adj, h) + ba
        z = np.maximum(a @ Wz + h @ Uz + bz, 0.0)
        r = np.maximum(a @ Wr + h @ Ur + br, 0.0)
        hc = np.tanh(a @ Wh + (r * h) @ Uh + bh) * mask
        h = (1.0 - z) * h + z * hc
    return np.asarray(h, dtype=np.float32)


def kernel(**inputs) -> np.ndarray:
    global LAST_RESULTS
    from concourse.bass_utils import run_bass_kernel_spmd

    x = np.asarray(inputs["x"], dtype=np.float32)
    adj = np.asarray(inputs["adj"], dtype=np.float32)
    mask = np.asarray(inputs["mask"], dtype=np.float32)
    steps = int(np.asarray(inputs["steps"]))
    biases = [np.asarray(inputs[k], dtype=np.float32)
              for k in ["b_enc", "bz", "br", "bh", "ba"]]

    if steps != 2 or any(np.any(b != 0.0) for b in biases) or np.any(mask != 1.0):
        # off-spec shape of the problem: bit-faithful host fallback
        return _fallback(
            x, adj, mask,
            *[np.asarray(inputs[k], np.float32) for k in
              ["W_enc", "b_enc", "Wz", "Uz", "bz", "Wr", "Ur", "br",
               "Wh", "Uh", "bh", "ba"]], steps)

    Ws = {k: np.asarray(inputs[k], dtype=np.float32)
          for k in ["W_enc", "Wz", "Uz", "Wr", "Ur", "Wh", "Uh"]}

    wmap = {}
    for (name, key, scale, want_lo) in [
            ("wenc", "W_enc", S_WENC, True),
            ("wz", "Wz", S_WZ, True),
            ("wr", "Wr", S_WR, True),
            ("wh1", "Wh", S_WH1, True),
            ("uh1", "Uh", S_UH1, True),
            ("uz1", "Uz", S_UZ1, False),
            ("ur1", "Ur", S_UR1, False),
            ("uz2", "Uz", S_UZ2, False),
            ("ur2", "Ur", S_UR2, False),
            ("wh2", "Wh", S_WH2, False),
            ("uh2", "Uh", S_UH2, False)]:
        hi, lo = _split8(Ws[key], scale)
        wmap[name + "hi"] = _lay(hi)
        if want_lo:
            wmap[name + "lo"] = _lay(lo)

    nc = _get()
    in_maps = []
    for c in range(NCORES):
        sl = slice(c * B_PC, (c + 1) * B_PC)
        xhi, xlo = _split8(x[sl], S_X)
        adjhi, adjlo = _split8(adj[sl], S_ADJ)
        in_maps.append({
            "xhi": _lay_batch(xhi), "xlo": _lay_batch(xlo),
            "adjhi": _lay_batch(adjhi), "adjlo": _lay_batch(adjlo),
            **wmap,
        })

    res = run_bass_kernel_spmd(nc, in_maps, core_ids=list(range(NCORES)))
    LAST_RESULTS = res
    out = np.concatenate([np.asarray(res.results[c]["out"]).transpose(0, 2, 1)
                          for c in range(NCORES)], axis=0)
    return np.ascontiguousarray(out)

